# revision 6
# baseline (speedup 1.0000x reference)
"""Trainium2 Bass kernel for nn_CrossModalAttention (B=32768, D=1024, H=16, OUT=3).

Math notes (exact simplifications of the reference):
  - Attention is over a single key (seq len 1) -> softmax == 1.0 exactly, so the
    attention output is just v @ wo.T + bo with v = xkv @ wv.T + bv.
  - (xkv @ wv.T + bv) @ wo.T + bo == xkv @ (wo @ wv).T + (wo @ bv + bo): each
    block collapses to ONE [B,D]x[D,D] matmul (weights fused on host in fp64).
  - ln_g/ln_b are folded into the fc1 weights/bias (exact algebra), so the
    layernorm on device is a pure normalize: y = (u - mean) * rsqrt(var + eps).
  - fc1 of the concat [ta|tv|av] splits into 3 per-block matmuls accumulated
    in PSUM.

Layout: feature-major ("transposed") activations throughout -> the contraction
dim of every matmul lies on SBUF partitions and no on-chip transposes are
needed.  LN stats (sums over features = partitions) via ones-vector matmuls on
the PE; per-sample stats broadcast back across partitions via K=1 matmuls.

Data parallel over 8 cores: batch 32768 -> 8 x 4096, weights replicated.
Matmuls run in float32r (fp32 storage, reduced-precision PE mode, 1 cyc/row at
free-dim >= 256; measured matmul rel-err ~1.5e-4 vs fp64, ~20x better than
bf16 at the same speed).
"""

import os
import sys

sys.path.insert(0, "/opt/trn_rl_repo")

import numpy as np

import concourse.bass as bass
import concourse.mybir as mybir
import concourse.tile as tile
from concourse import bacc
from concourse.bass_utils import run_bass_kernel_spmd

F32 = mybir.dt.float32
F32R = mybir.dt.float32r
ADD = mybir.AluOpType.add
SUB = mybir.AluOpType.subtract
MUL = mybir.AluOpType.mult
ACT_F = mybir.ActivationFunctionType

B, D, OUT = 32768, 1024, 3
NCORES = 8
LN_EPS = 1e-5
JT = D // 128  # 8 feature tiles
KT = D // 128  # 8 contraction tiles

_cache: dict = {}


def _build(b_loc: int, bc: int):
    """Build + compile the per-core Bass module (SPMD, same on all cores)."""
    nch = b_loc // bc
    nc = bacc.Bacc(None, target_bir_lowering=False, debug=False)

    # ---- DRAM I/O ----
    xt_d = nc.dram_tensor("xt", [128, KT, b_loc], F32R, kind="ExternalInput")
    xa_d = nc.dram_tensor("xa", [128, KT, b_loc], F32R, kind="ExternalInput")
    xv_d = nc.dram_tensor("xv", [128, KT, b_loc], F32R, kind="ExternalInput")
    # fused attention weights, tiled [i, j, p, k, jcol]
    wtt_d = nc.dram_tensor("wtt", [3, JT, 128, KT, 128], F32R, kind="ExternalInput")
    # fc1 (ln-folded) weights, tiled [i, j2, p, k, hcol]
    gtt_d = nc.dram_tensor("gtt", [3, JT, 128, KT, 128], F32R, kind="ExternalInput")
    bsw_d = nc.dram_tensor("bsw", [3, 128, JT], F32, kind="ExternalInput")
    f1b_d = nc.dram_tensor("f1b", [128, JT], F32, kind="ExternalInput")
    f2t_d = nc.dram_tensor("f2t", [128, KT, OUT], F32R, kind="ExternalInput")
    f2b_d = nc.dram_tensor("f2b", [OUT, 1], F32, kind="ExternalInput")
    onec_d = nc.dram_tensor("onec", [128, 1], F32R, kind="ExternalInput")
    oner_d = nc.dram_tensor("oner", [1, 128], F32R, kind="ExternalInput")
    out_d = nc.dram_tensor("outT", [OUT, b_loc], F32, kind="ExternalOutput")

    blocks = [("t", "a"), ("t", "v"), ("a", "v")]  # (query/residual, key-value)

    with tile.TileContext(nc) as tc:
        with (
            tc.tile_pool(name="const", bufs=1) as const,
            tc.tile_pool(name="xp", bufs=3) as xp,
            tc.tile_pool(name="up", bufs=2) as up,
            tc.tile_pool(name="yp", bufs=3) as yp,
            tc.tile_pool(name="hp", bufs=1) as hp,
            tc.tile_pool(name="wp", bufs=4) as wp,
            tc.tile_pool(name="gp", bufs=4) as gp,
            tc.tile_pool(name="op", bufs=2) as op_pool,
            tc.tile_pool(name="srow", bufs=2) as srow,
            tc.tile_pool(name="mmps", bufs=3, space="PSUM") as mmps,
            tc.tile_pool(name="stps", bufs=1, space="PSUM") as stps,
            tc.tile_pool(name="bcps", bufs=1, space="PSUM") as bcps,
            tc.tile_pool(name="ops", bufs=1, space="PSUM") as ops,
        ):
            ones_col = const.tile([128, 1], F32R, tag="ones_col")
            nc.sync.dma_start(ones_col, onec_d[:, :])
            ones_row = const.tile([1, 128], F32R, tag="ones_row")
            nc.sync.dma_start(ones_row, oner_d[:, :])
            eps_sb = const.tile([1, 1], F32, tag="eps")
            nc.vector.memset(eps_sb, LN_EPS)
            b_sb = []
            for i in range(3):
                t = const.tile([128, JT], F32, tag=f"b{i}")
                nc.sync.dma_start(t, bsw_d[i, :, :])
                b_sb.append(t)
            f1b_sb = const.tile([128, JT], F32, tag="f1b")
            nc.sync.dma_start(f1b_sb, f1b_d[:, :])
            f2t_sb = const.tile([128, KT, OUT], F32R, tag="f2t")
            nc.sync.dma_start(f2t_sb, f2t_d[:, :, :])
            f2b_sb = const.tile([OUT, 1], F32, tag="f2b")
            nc.sync.dma_start(f2b_sb, f2b_d[:, :])

            for c in range(nch):
                cs = slice(c * bc, (c + 1) * bc)
                x_sb = {}
                for mod, dram in (("t", xt_d), ("a", xa_d), ("v", xv_d)):
                    t = xp.tile([128, KT, bc], F32R, tag="x")
                    nc.sync.dma_start(t, dram[:, :, cs])
                    x_sb[mod] = t

                # ---- phase 1: three attention blocks, software-pipelined ----
                ys = [None, None, None]
                state = [None, None, None]  # (u_sb, y_sb, s1, s2)

                def u_phase(i):
                    qm, km = blocks[i]
                    xq, xkv = x_sb[qm], x_sb[km]
                    u_sb = up.tile([128, JT, bc], F32R, tag="u")
                    y_sb = yp.tile([128, JT, bc], F32R, tag="y")
                    for j in range(JT):
                        w_sb = wp.tile([128, KT, 128], F32R, tag="w")
                        nc.sync.dma_start(w_sb, wtt_d[i, j, :, :, :])
                        u_ps = mmps.tile([128, bc], F32, tag="mm")
                        for k in range(KT):
                            nc.tensor.matmul(
                                u_ps, w_sb[:, k, :], xkv[:, k, :],
                                start=(k == 0), stop=(k == KT - 1),
                            )
                        # evacuate PSUM + bias + residual in one DVE pass
                        nc.vector.scalar_tensor_tensor(
                            out=u_sb[:, j, :], in0=u_ps,
                            scalar=b_sb[i][:, j : j + 1], in1=xq[:, j, :],
                            op0=ADD, op1=ADD,
                        )
                        # square into y (scratch until normalize overwrites it)
                        nc.scalar.activation(y_sb[:, j, :], u_sb[:, j, :], ACT_F.Square)
                    state[i] = (u_sb, y_sb)

                def stats_and_norm(i):
                    u_sb, y_sb = state[i]
                    s1 = stps.tile([1, bc], F32, tag="s1")
                    s2 = stps.tile([1, bc], F32, tag="s2")
                    for j in range(JT):
                        nc.tensor.matmul(
                            s1, ones_col, u_sb[:, j, :],
                            start=(j == 0), stop=(j == JT - 1),
                        )
                        nc.tensor.matmul(
                            s2, ones_col, y_sb[:, j, :],
                            start=(j == 0), stop=(j == JT - 1),
                        )
                    m_sb = srow.tile([1, bc], F32, tag="m")
                    nc.scalar.mul(m_sb, s1, 1.0 / D)
                    msq = srow.tile([1, bc], F32, tag="msq")
                    nc.vector.tensor_mul(msq, m_sb, m_sb)
                    var = srow.tile([1, bc], F32, tag="var")
                    nc.vector.scalar_tensor_tensor(
                        out=var, in0=s2, scalar=1.0 / D, in1=msq, op0=MUL, op1=SUB
                    )
                    std = srow.tile([1, bc], F32, tag="std")
                    nc.scalar.activation(std, var, ACT_F.Sqrt, bias=eps_sb, scale=1.0)
                    r_sb = srow.tile([1, bc], F32R, tag="r")
                    mr_sb = srow.tile([1, bc], F32R, tag="mr")
                    with nc.allow_low_precision(reason="f32r rounding of LN scale ~1e-5"):
                        nc.vector.reciprocal(r_sb, std)
                        nc.vector.tensor_mul(mr_sb, m_sb, r_sb)
                    rbc = bcps.tile([128, bc], F32, tag="rbc")
                    nc.tensor.matmul(rbc, ones_row, r_sb, start=True, stop=True)
                    mrbc = bcps.tile([128, bc], F32, tag="mrbc")
                    nc.tensor.matmul(mrbc, ones_row, mr_sb, start=True, stop=True)
                    for j in range(JT):
                        nc.vector.tensor_mul(y_sb[:, j, :], u_sb[:, j, :], rbc)
                        nc.vector.tensor_sub(y_sb[:, j, :], y_sb[:, j, :], mrbc)
                    ys[i] = y_sb

                # pipeline: hide block i's stats latency under block i+1 matmuls
                u_phase(0)
                u_phase(1)
                stats_and_norm(0)
                u_phase(2)
                stats_and_norm(1)
                stats_and_norm(2)

                # ---- phase 2: fc1 (3-way accumulate) + relu, then fc2 ----
                h_sb = hp.tile([128, JT, bc], F32R, tag="h")
                for j2 in range(JT):
                    z_ps = mmps.tile([128, bc], F32, tag="mm")
                    for i in range(3):
                        g_sb = gp.tile([128, KT, 128], F32R, tag="g")
                        nc.sync.dma_start(g_sb, gtt_d[i, j2, :, :, :])
                        for k in range(KT):
                            nc.tensor.matmul(
                                z_ps, g_sb[:, k, :], ys[i][:, k, :],
                                start=(i == 0 and k == 0),
                                stop=(i == 2 and k == KT - 1),
                            )
                    nc.scalar.activation(
                        h_sb[:, j2, :], z_ps, ACT_F.Relu,
                        bias=f1b_sb[:, j2 : j2 + 1], scale=1.0,
                    )
                o_ps = ops.tile([OUT, bc], F32, tag="o")
                for k in range(KT):
                    nc.tensor.matmul(
                        o_ps, f2t_sb[:, k, :], h_sb[:, k, :],
                        start=(k == 0), stop=(k == KT - 1),
                    )
                o_sb = op_pool.tile([OUT, bc], F32, tag="osb")
                nc.scalar.activation(
                    o_sb, o_ps, ACT_F.Identity, bias=f2b_sb, scale=1.0
                )
                nc.sync.dma_start(out_d[:, cs], o_sb)

    nc.compile()
    return nc


def _swizzle_weight(wt: np.ndarray) -> np.ndarray:
    """[D, D] lhsT matrix (contraction-major) -> [JT, 128, KT, 128] tiles where
    tile[j][p, k, jc] = wt[k*128+p, j*128+jc] (per-partition-contiguous)."""
    d = wt.shape[0]
    jt, kt = d // 128, wt.shape[0] // 128
    # wt[(k p), (j jc)] -> [j, p, k, jc]
    return np.ascontiguousarray(
        wt.reshape(kt, 128, jt, 128).transpose(2, 1, 0, 3)
    )


def _swizzle_x(shard: np.ndarray) -> np.ndarray:
    """[b_loc, D] activation shard -> feature-major [128, KT, b_loc]."""
    b_loc = shard.shape[0]
    return np.ascontiguousarray(shard.reshape(b_loc, KT, 128).transpose(2, 1, 0))


def _prep_shared(w_qkv, b_qkv, w_o, b_o, ln_g, ln_b, fc1_w, fc1_b, fc2_w, fc2_b):
    f6 = np.float64
    wtt = np.empty((3, JT, 128, KT, 128), np.float32)
    bsw = np.empty((3, 128, JT), np.float32)
    gtt = np.empty((3, JT, 128, KT, 128), np.float32)
    f1b_full = fc1_b.astype(f6).copy()
    for i in range(3):
        wv, bv = w_qkv[i, 2].astype(f6), b_qkv[i, 2].astype(f6)
        wo, bo = w_o[i].astype(f6), b_o[i].astype(f6)
        w_i = wo @ wv                      # [j_out, d_in]
        bias_i = wo @ bv + bo              # [j_out]
        wtt[i] = _swizzle_weight(np.ascontiguousarray(w_i.T).astype(np.float32))
        bsw[i] = bias_i.astype(np.float32).reshape(JT, 128).T
        f_i = fc1_w[:, i * D : (i + 1) * D].astype(f6)   # [h, j]
        g_i = f_i * ln_g[i].astype(f6)[None, :]
        f1b_full += f_i @ ln_b[i].astype(f6)
        gtt[i] = _swizzle_weight(np.ascontiguousarray(g_i.T).astype(np.float32))
    f1bsw = f1b_full.astype(np.float32).reshape(JT, 128).T.copy()
    f2t = np.ascontiguousarray(fc2_w.astype(np.float32).T)     # [D, OUT]
    f2sw = np.ascontiguousarray(f2t.reshape(KT, 128, OUT).transpose(1, 0, 2))
    f2bv = fc2_b.astype(np.float32).reshape(OUT, 1)
    return dict(
        wtt=wtt, gtt=gtt, bsw=bsw, f1b=np.ascontiguousarray(f1bsw),
        f2t=f2sw, f2b=f2bv,
        onec=np.ones((128, 1), np.float32), oner=np.ones((1, 128), np.float32),
    )


def kernel(
    text_x, audio_x, video_x, w_qkv, b_qkv, w_o, b_o, ln_g, ln_b,
    fc1_w, fc1_b, fc2_w, fc2_b, num_heads=16,
):
    text_x = np.asarray(text_x, np.float32)
    audio_x = np.asarray(audio_x, np.float32)
    video_x = np.asarray(video_x, np.float32)
    b_total = text_x.shape[0]
    b_loc = b_total // NCORES
    bc = min(512, b_loc)

    key = (b_loc, bc)
    if key not in _cache:
        _cache[key] = _build(b_loc, bc)
    nc = _cache[key]

    shared = _prep_shared(
        np.asarray(w_qkv), np.asarray(b_qkv), np.asarray(w_o), np.asarray(b_o),
        np.asarray(ln_g), np.asarray(ln_b), np.asarray(fc1_w),
        np.asarray(fc1_b), np.asarray(fc2_w), np.asarray(fc2_b),
    )
    in_maps = []
    for cidx in range(NCORES):
        sl = slice(cidx * b_loc, (cidx + 1) * b_loc)
        in_maps.append(
            dict(
                xt=_swizzle_x(text_x[sl]),
                xa=_swizzle_x(audio_x[sl]),
                xv=_swizzle_x(video_x[sl]),
                **shared,
            )
        )

    res = run_bass_kernel_spmd(nc, in_maps, core_ids=list(range(NCORES)))
    out = np.empty((b_total, OUT), np.float32)
    for cidx in range(NCORES):
        sl = slice(cidx * b_loc, (cidx + 1) * b_loc)
        out[sl] = res.results[cidx]["outT"].T
    return out
